# revision 34
# baseline (speedup 1.0000x reference)
"""Trainium2 Bass kernel for nn_DVQuantumLayer (12-qubit, 2-layer variational
circuit, batch 512), data-parallel over 8 NeuronCores (64 samples each).

Method: the circuit is algebraically compiled into 4 dense "phase" operators
plus a product-state embedding and a sign-contraction measurement, all
executed as float32r TensorE matmuls on a [128 x 2048] complex statevector
laid out as (r|i) planes of a [128, 4096] SBUF tile.

State index: z = h*32 + l with h = qubits 0-6 (q0 MSB), l = qubits 7-11.
Two alternating layouts (b = b16*4 + b4, 64 samples/core):
  B': [p = l*4+b4,  f = b16*128 + h]
  A : [p = h,       f = b16*128 + l*4 + b4]
Each phase operator has the form  M1 (x) I + M2 (x) Xflip  where M1/M2 act on
the partition side and Xflip flips f-bit-64 within each 128-column chunk.
Applying a phase = 16 chunks of accumulating matmuls with the state chunk as
the stationary operand; the output lands transposed, i.e. already in the
other layout. Phases 0-2 are stored diagonalized in the Hadamard basis of
their Xflip bit (M1 (x) I + M2 (x) X = H (M1+M2 (+) M1-M2) H, every H folded
into an adjacent host-side table, into s0, or into ph3's rows), so they need
no flipped state copies; only ph3's Xflip term uses a flipped copy of ph2's
output, written by the otherwise-idle GPSIMD engine.

The timing-rep loop is a hardware loop (tc.For_i) running 16 software-
pipelined pipelines per trip (each pipeline's measure matmuls are emitted
after the next pipeline's phase matmuls; the next initial state prefetches
under the current phases), so a reps=R NEFF has the same instruction count
as reps=16 and the harness delta measures steady-state device time per
pipeline iteration.
"""

import sys

sys.path.insert(0, "/opt/trn_rl_repo")

import numpy as np

import concourse.bacc as bacc
import concourse.mybir as mybir
from concourse.ap import AP
from concourse.bass_utils import run_bass_kernel_spmd
from concourse.tile import TileContext

NQ = 12
NL = 2
B = 512
NCORES = 8
BC = B // NCORES  # 64

F32 = mybir.dt.float32
F32R = mybir.dt.float32r
C128 = np.complex128

# ----------------------------------------------------------------------------
# Host-side math: gate matrices -> phase operators -> packed device tables
# ----------------------------------------------------------------------------


def _rx(t):
    c, s = np.cos(t / 2), np.sin(t / 2)
    return np.array([[c, -1j * s], [-1j * s, c]], dtype=C128)


def _rz(t):
    e = np.exp(-0.5j * t)
    return np.array([[e, 0], [0, np.conj(e)]], dtype=C128)


def _crx(t):
    m = np.eye(4, dtype=C128)
    m[2:, 2:] = _rx(t)
    return m


def _op_2q(G, qa, qb, n):
    dim = 2**n
    M = np.zeros((dim, dim), dtype=C128)
    sa, sb = 1 << (n - 1 - qa), 1 << (n - 1 - qb)
    for z in range(dim):
        a, b = (z // sa) % 2, (z // sb) % 2
        base = z - a * sa - b * sb
        for a2 in range(2):
            for b2 in range(2):
                M[base + a2 * sa + b2 * sb, z] += G[a2 * 2 + b2, a * 2 + b]
    return M


def _kron_list(mats):
    M = np.array([[1.0 + 0j]])
    for m in mats:
        M = np.kron(M, m)
    return M


def _build_tables(params):
    params = np.asarray(params, dtype=np.float64)
    oneq = [
        [_rz(params[l, NQ + q]) @ _rx(params[l, q]) for q in range(NQ)]
        for l in range(NL)
    ]
    lidx = np.arange(32)
    P0 = np.diag((1 - (lidx & 1)).astype(C128))
    P1 = np.diag((lidx & 1).astype(C128))
    hidx = np.arange(128)
    P0h = np.diag((1 - (hidx & 1)).astype(C128))
    P1h = np.diag((hidx & 1).astype(C128))

    def chainB(p):
        M = np.eye(32, dtype=C128)
        for (c, t), th in [((3, 4), p[1]), ((2, 3), p[2]), ((1, 2), p[3]),
                           ((0, 1), p[4])]:
            M = _op_2q(_crx(th), c, t, 5) @ M
        return M

    def chainA(p):
        M = np.eye(128, dtype=C128)
        for (c, t), th in [((5, 6), p[6]), ((4, 5), p[7]), ((3, 4), p[8]),
                           ((2, 3), p[9]), ((1, 2), p[10]), ((0, 1), p[11])]:
            M = _op_2q(_crx(th), c, t, 7) @ M
        return M

    phases = []
    for l in range(NL):
        p = params[l]
        c0, s0 = np.cos(p[0] / 2), np.sin(p[0] / 2)
        c5, s5 = np.cos(p[5] / 2), np.sin(p[5] / 2)
        cB = chainB(p)
        F1 = cB @ (P0 + c0 * P1)
        F2 = -1j * s0 * (cB @ P1)
        if l > 0:
            ol = _kron_list(oneq[l][7:])
            F1, F2 = F1 @ ol, F2 @ ol
        cA = chainA(p)
        E1 = cA @ (P0h + c5 * P1h)
        E2 = -1j * s5 * (cA @ P1h)
        if l + 1 < NL:
            oh = _kron_list(oneq[l + 1][:7])
            E1, E2 = oh @ E1, oh @ E2
        phases.append(("B", F1, F2))
        phases.append(("A", E1, E2))
    return oneq, phases


def _embed_factors(x, oneq):
    x = np.asarray(x, dtype=np.float64)
    nb = x.shape[0]
    u = np.empty((nb, NQ, 2), dtype=C128)
    for q in range(NQ):
        v = np.stack([np.cos(x[:, q] / 2), -1j * np.sin(x[:, q] / 2)], axis=1)
        u[:, q] = v @ oneq[0][q].T
    a = u[:, 0]
    for q in range(1, 7):
        a = np.einsum("bi,bj->bij", a, u[:, q]).reshape(nb, -1)
    c = u[:, 7]
    for q in range(8, 12):
        c = np.einsum("bi,bj->bij", c, u[:, q]).reshape(nb, -1)
    return a, c


def _bf(A, axis):
    """Butterfly (unnormalized Hadamard pairing) on bit-64 of a 128 axis."""
    s2 = np.sqrt(0.5)
    A = np.moveaxis(A, axis, 0)
    out = np.concatenate([(A[0:64] + A[64:128]) * s2,
                          (A[0:64] - A[64:128]) * s2], axis=0)
    return np.moveaxis(out, 0, axis)


def _shared_tables(params):
    """Phase + measurement tables (identical on all cores).

    Phases 0-2 are stored diagonalized in the Hadamard basis of their
    Xflip bit:  M1 (x) I + M2 (x) X  =  H (M1+M2 (+) M1-M2) H,  with every
    H factor folded into an adjacent table (rows = post-H of the previous
    phase, cols = pre-H of the next) or into s0 / the classic ph3 rows.
    The device then needs no flipped state copies for ph0-ph2."""
    _, phases = _build_tables(params)
    out = {}

    def kron4(M):
        return np.kron(M.T, np.eye(4))

    G = []
    for i, (side, M1, M2) in enumerate(phases):
        GP = kron4(M1 + M2) if side == "B" else (M1 + M2).T
        GM = kron4(M1 - M2) if side == "B" else (M1 - M2).T
        G.append([GP, GM])
    G[0] = [_bf(g, 1) for g in G[0]]                 # pre-H1 on cols
    G[1] = [_bf(_bf(g, 0), 1) for g in G[1]]         # post-H0 rows, pre-H2 cols
    G[2] = [_bf(_bf(g, 0), 1) for g in G[2]]         # post-H1 rows, pre-H3 cols
    G[3] = [_bf(g, 0) for g in G[3]]                 # post-H2 rows; post-H3
    # lands on the final partition bit (qubit 7) and is folded into the
    # measurement via the |chi0|^2+|chi1|^2 / Re(chi0 chi1*) split

    for i, (Ga, Gb) in enumerate(G):
        R0 = np.concatenate([Ga.real, Ga.imag], 1)
        R1 = np.concatenate([-Ga.imag, Ga.real], 1)
        R2 = np.concatenate([Gb.real, Gb.imag], 1)
        R3 = np.concatenate([-Gb.imag, Gb.real], 1)
        out[f"ph{i}"] = np.ascontiguousarray(
            np.concatenate([R0, R1, R2, R3], 1), dtype=np.float32)

    V = np.zeros((128, 24), np.float32)
    lv, b4v = np.arange(128) // 4, np.arange(128) % 4
    for b4 in range(4):
        V[b4v == b4, b4] = 1.0
    for jq in range(5):
        sig = 1 - 2 * ((lv >> (4 - jq)) & 1)
        for b4 in range(4):
            V[b4v == b4, 4 + jq * 4 + b4] = sig[b4v == b4]
    Vq = (V[0:64] + V[64:128]) * 0.5
    out["mV"] = np.ascontiguousarray(np.concatenate([Vq, Vq], 0))
    out["mVx"] = np.ascontiguousarray(V[0:64] - V[64:128])
    W = np.zeros((128, 10), np.float32)
    hv = np.arange(128)
    for q in range(7):
        W[:, q] = 1 - 2 * ((hv >> (6 - q)) & 1)
    W[:, 8] = 1.0
    W[:, 9] = 1.0
    out["mW"] = W
    return out


def _core_tables(x_shard, oneq):
    """Per-core initial product state (64 samples) in B' layout:
    s0[l*4+b4, b16*128+h] with the (r|i) planes side by side."""
    a, c = _embed_factors(x_shard, oneq)
    full = a[:, :, None] * c[:, None, :]          # [64, h=128, l=32]
    arr = full.reshape(16, 4, 128, 32)            # [b16, b4, h, l]
    perm = arr.transpose(3, 1, 0, 2).reshape(128, 2048)
    s0 = np.empty((128, 4096), np.float32)
    s0[:, 0:2048] = perm.real
    s0[:, 2048:4096] = perm.imag
    # pre-H0: butterfly the passive h-bit-64 within each 128-col chunk
    v = s0.reshape(128, 2, 16, 2, 64)
    s2 = np.sqrt(0.5, dtype=np.float32)
    w = np.empty_like(v)
    w[:, :, :, 0, :] = (v[:, :, :, 0, :] + v[:, :, :, 1, :]) * s2
    w[:, :, :, 1, :] = (v[:, :, :, 0, :] - v[:, :, :, 1, :]) * s2
    s0 = np.ascontiguousarray(w.reshape(128, 4096))
    return {"s0": s0}


# ----------------------------------------------------------------------------
# Device program
# ----------------------------------------------------------------------------


def _evac_dst(tile, g):
    """[128,256] PSUM -> plane-split chunk g of a [128,4096] state tile."""
    return AP(tensor=tile.tensor, offset=g * 128,
              ap=[[4096, 128], [2048, 2], [1, 128]])


def _chunk(tile, g, plane):
    """Stationary AP for chunk g, plane 0/1 of a state tile."""
    off = plane * 2048 + g * 128
    return AP(tensor=tile.tensor, offset=off,
              ap=[[4096, 128], [1, 128]])


def _chunk_h(tile, g, plane, half):
    """64-col half of chunk g (the +/- Hadamard subspace)."""
    off = plane * 2048 + g * 128 + half * 64
    return AP(tensor=tile.tensor, offset=off,
              ap=[[4096, 128], [1, 64]])


def _flip_dst(tile, g):
    """Chunk g of a state tile, written at f-bit-64-flipped positions."""
    return AP(tensor=tile.tensor, offset=g * 128 + 64,
              ap=[[4096, 128], [2048, 2], [-64, 2], [1, 64]])


def build_nc(reps=1):
    nc = bacc.Bacc("TRN2", target_bir_lowering=False)
    d = {}
    for name, shape in [("s0", [128, 4096]),
                        ("ph0", [128, 1024]), ("ph1", [128, 1024]),
                        ("ph2", [128, 1024]), ("ph3", [128, 1024]),
                        ("mV", [128, 24]), ("mVx", [64, 24]),
                        ("mW", [128, 10])]:
        d[name] = nc.dram_tensor(name, shape, F32R, kind="ExternalInput")
    y_d = nc.dram_tensor("y", [BC, NQ], F32, kind="ExternalOutput")

    with TileContext(nc) as tc:
        with (
            tc.tile_pool(name="tabs", bufs=1) as tabs,
            tc.tile_pool(name="st", bufs=3) as stp,
            tc.tile_pool(name="s0p", bufs=2) as s0p,
            tc.tile_pool(name="ms", bufs=2) as msp,
            tc.tile_pool(name="pp", bufs=4, space="PSUM") as pp,
            tc.tile_pool(name="pm", bufs=2, space="PSUM") as pm,
        ):
            tt = {}
            for name in ("ph0", "ph1", "ph2", "ph3", "mV", "mVx", "mW"):
                tile = tabs.tile(list(d[name].shape), d[name].dtype, tag=name)
                nc.sync.dma_start(out=tile, in_=d[name].ap())
                tt[name] = tile

            # PSUM evacuation alternates the two PSUM-capable copy engines
            # (GPSIMD cannot access PSUM); flip copies run SBUF->SBUF on it
            evac_engs = (nc.scalar.copy, nc.vector.tensor_copy)

            # Warm the ACT function tables (Square + Copy) before the loop so
            # the act-table-load pass sees them loaded on every path into the
            # body and does not reload (1.3us) each iteration.
            warm = msp.tile([128, 16], F32, tag="warm", bufs=1)
            nc.scalar.activation(
                out=warm[:, 0:8], in_=tt["mW"][:, 0:8].bitcast(F32),
                func=mybir.ActivationFunctionType.Square)
            nc.scalar.copy(out=warm[:, 8:16], in_=tt["mW"][:, 0:8])

            def phases_part(ecnt):
                # ---- initial state: uneven plane-paired DMA segments (the
                # first covers just ph0's first chunk group) so the first
                # matmul starts early; S sits in a dedicated 2-buffer pool
                # whose previous buffer was last read 3 pipelines ago, so
                # the next pipeline's load prefetches under this one's
                # phases. ph0-ph2 are diagonal in the folded Hadamard basis
                # and need no flipped state; only ph3's Xflip term does.
                S = s0p.tile([128, 4096], F32R, tag="s0")
                segs = [(0, 256), (256, 768), (1024, 512), (1536, 512)]
                for off, width in segs:
                    seg = AP(tensor=S.tensor, offset=off,
                             ap=[[4096, 128], [2048, 2], [1, width]])
                    src_seg = AP(tensor=d["s0"], offset=off,
                                 ap=[[4096, 128], [2048, 2], [1, width]])
                    nc.sync.dma_start(out=seg, in_=src_seg)

                # ---- 4 phases, all diagonal in the folded H basis.
                # ph3's minus-half PSUM is additionally evacuated to a
                # base-partition-0 tile `lo` so the q7 cross term can be
                # built with legal equal-base elementwise ops.
                lo = msp.tile([64, 4096], F32R, tag="lo")
                for pi in range(4):
                    tab = tt[f"ph{pi}"]
                    S2 = stp.tile([128, 4096], F32R, tag="st")
                    for gp in range(8):
                        psP = pp.tile([64, 512], F32, tag="ppP", bufs=3)
                        psM = pp.tile([64, 512], F32, tag="ppM", bufs=3)
                        for half in range(2):
                            g = gp * 2 + half
                            for hb, ph in ((0, psP), (1, psM)):
                                poh = ph[:, half * 256:(half + 1) * 256]
                                nc.tensor.matmul(
                                    poh, _chunk_h(S, g, 0, hb),
                                    tab[:, hb * 512:hb * 512 + 256],
                                    start=True, stop=False)
                                nc.tensor.matmul(
                                    poh, _chunk_h(S, g, 1, hb),
                                    tab[:, hb * 512 + 256:hb * 512 + 512],
                                    start=False, stop=True)
                        for hb, ph in ((0, psP), (1, psM)):
                            esrc = ph.rearrange("p (c a j) -> p c a j",
                                                c=2, a=2)
                            edst = AP(tensor=S2.tensor,
                                      offset=hb * 64 * 4096 + gp * 256,
                                      ap=[[4096, 64], [128, 2],
                                          [2048, 2], [1, 128]])
                            evac_engs[ecnt % 2](out=edst, in_=esrc)
                            ecnt += 1
                        if pi == 3:
                            lsrc = psM.rearrange("p (c a j) -> p c a j",
                                                 c=2, a=2)
                            ldst = AP(tensor=lo.tensor, offset=gp * 256,
                                      ap=[[4096, 64], [128, 2],
                                          [2048, 2], [1, 128]])
                            evac_engs[ecnt % 2](out=ldst, in_=lsrc)
                            ecnt += 1
                    S = S2
                return S, lo, ecnt

            def square_part(S, lo):
                # probs = re^2 + im^2 (ACT/DVE/Pool) and the q7 cross term
                # C = re0*re1 + im0*im1 built from the base-0 copy `lo` of
                # the upper partition half, entirely on the idle Pool
                # engine; all run under the next pipeline's phase matmuls
                sq = msp.tile([128, 2048], F32R, tag="sq", name="sq_r")
                probs = msp.tile([128, 2048], F32R, tag="probs")
                Ct = msp.tile([64, 2048], F32R, tag="Ct", bufs=1)
                C = msp.tile([64, 2048], F32R, tag="C")
                for blk in range(4):
                    c0, c1 = blk * 512, (blk + 1) * 512
                    nc.scalar.activation(
                        out=sq[:, c0:c1],
                        in_=S[:, c0:c1].bitcast(F32),
                        func=mybir.ActivationFunctionType.Square)
                    nc.vector.tensor_mul(out=probs[:, c0:c1],
                                         in0=S[:, 2048 + c0:2048 + c1],
                                         in1=S[:, 2048 + c0:2048 + c1])
                    nc.gpsimd.tensor_add(out=probs[:, c0:c1],
                                         in0=probs[:, c0:c1],
                                         in1=sq[:, c0:c1])
                    nc.gpsimd.tensor_mul(out=Ct[:, c0:c1],
                                         in0=S[0:64, c0:c1],
                                         in1=lo[:, c0:c1])
                    nc.gpsimd.tensor_mul(out=C[:, c0:c1],
                                         in0=S[0:64, 2048 + c0:2048 + c1],
                                         in1=lo[:, 2048 + c0:2048 + c1])
                    nc.gpsimd.tensor_add(out=C[:, c0:c1],
                                         in0=C[:, c0:c1],
                                         in1=Ct[:, c0:c1])
                return probs, C

            def measure_part(pc, ecnt):
                probs, C = pc
                # S1 column layout: [0:64] = sel block (g*4+b4),
                # [64*(j+1) : 64*(j+2)] = low-qubit j block (g*4+b4), j=0..4
                S1 = msp.tile([128, 384], F32R, tag="S1")
                ps2 = pm.tile([64, 24], F32, tag="pm2", bufs=1)
                # one PSUM bank, 4 rotating 24-col regions (region deps are
                # tracked per AP range, so matmul g+4 waits on evac g)
                psA = pm.tile([128, 96], F32, tag="pm1", bufs=1)
                for g in range(16):
                    reg = psA[:, (g % 4) * 24:(g % 4) * 24 + 24]
                    nc.tensor.matmul(reg, probs[:, g * 128:(g + 1) * 128],
                                     tt["mV"], start=True, stop=False)
                    nc.tensor.matmul(reg, C[:, g * 128:(g + 1) * 128],
                                     tt["mVx"], start=False, stop=True)
                    # cols (j=sel,q7..q11; b4) -> S1[:, 64*j + g*4 + b4]
                    dall = S1.rearrange("p (j c) -> p j c", c=64)[
                        :, 0:6, g * 4:g * 4 + 4]
                    evac_engs[ecnt % 2](out=dall, in_=reg)
                    ecnt += 1
                # stage 2: everything lands as [b-rows, cols] in one PSUM
                nc.tensor.matmul(ps2[:, 0:8], S1[:, 0:64], tt["mW"][:, 0:8],
                                 start=True, stop=True)
                for j in range(5):
                    nc.tensor.matmul(ps2[:, 8 + 2 * j:10 + 2 * j],
                                     S1[:, 64 * (j + 1):64 * (j + 2)],
                                     tt["mW"][:, 8:10],
                                     start=True, stop=True)
                yt = msp.tile([64, 12], F32, tag="yt")
                nc.scalar.copy(out=yt[:, 0:7], in_=ps2[:, 0:7])
                lowsrc = ps2[:, 8:18].rearrange(
                    "p (a b) -> p a b", b=2)[:, :, 0:1]
                nc.vector.tensor_copy(out=yt[:, 7:12].unsqueeze(-1),
                                      in_=lowsrc)
                nc.sync.dma_start(out=y_d.ap(), in_=yt)
                return ecnt

            if reps == 1:
                S, lo, ecnt = phases_part(0)
                measure_part(square_part(S, lo), ecnt)
            else:
                # software-pipelined 16x-unrolled body: each pipeline's
                # measure matmuls are emitted after the NEXT pipeline's phase
                # matmuls so the PE never waits on square/evac chains, and
                # the loop barrier + first-pipeline DMA exposure amortize
                # over 16 pipelines
                assert reps % 16 == 0, "looped NEFF runs 16 pipelines per trip"
                with tc.For_i(0, reps // 16):
                    probs_prev = None
                    ecnt = 0
                    for k in range(16):
                        S, lo, ecnt = phases_part(ecnt)
                        if probs_prev is not None:
                            ecnt = measure_part(probs_prev, ecnt)
                        probs_prev = square_part(S, lo)
                    measure_part(probs_prev, ecnt)

    nc.compile()
    return nc


_NC_CACHE = {}


def _get_nc(reps=1):
    if reps not in _NC_CACHE:
        _NC_CACHE[reps] = build_nc(reps)
    return _NC_CACHE[reps]


def make_in_maps(x, params):
    oneq, _ = _build_tables(params)
    shared = _shared_tables(params)
    in_maps = []
    for core in range(NCORES):
        m = dict(shared)
        m.update(_core_tables(x[core * BC:(core + 1) * BC], oneq))
        in_maps.append(m)
    return in_maps


def kernel(x, params, _reps=1, _nc=None):
    x = np.asarray(x)
    params = np.asarray(params)
    nc = _nc if _nc is not None else _get_nc(_reps)
    in_maps = make_in_maps(x, params)
    res = run_bass_kernel_spmd(nc, in_maps, list(range(NCORES)))
    return np.concatenate(
        [res.results[c]["y"] for c in range(NCORES)], axis=0
    ).astype(np.float32)


# revision 35
# speedup vs baseline: 1.0928x; 1.0928x over previous
"""Trainium2 Bass kernel for nn_DVQuantumLayer (12-qubit, 2-layer variational
circuit, batch 512), data-parallel over 8 NeuronCores (64 samples each).

Method: the circuit is algebraically compiled into 4 dense "phase" operators
plus a product-state embedding and a sign-contraction measurement, all
executed as float32r TensorE matmuls on a [128 x 2048] complex statevector
laid out as (r|i) planes of a [128, 4096] SBUF tile.

State index: z = h*32 + l with h = qubits 0-6 (q0 MSB), l = qubits 7-11.
Two alternating layouts (b = b16*4 + b4, 64 samples/core):
  B': [p = l*4+b4,  f = b16*128 + h]
  A : [p = h,       f = b16*128 + l*4 + b4]
Each phase operator has the form  M1 (x) I + M2 (x) Xflip  where M1/M2 act on
the partition side and Xflip flips f-bit-64 within each 128-column chunk.
Applying a phase = 16 chunks of accumulating matmuls with the state chunk as
the stationary operand; the output lands transposed, i.e. already in the
other layout. Phases 0-2 are stored diagonalized in the Hadamard basis of
their Xflip bit (M1 (x) I + M2 (x) X = H (M1+M2 (+) M1-M2) H, every H folded
into an adjacent host-side table, into s0, or into ph3's rows), so they need
no flipped state copies; only ph3's Xflip term uses a flipped copy of ph2's
output, written by the otherwise-idle GPSIMD engine.

The timing-rep loop is a hardware loop (tc.For_i) running 16 software-
pipelined pipelines per trip (each pipeline's measure matmuls are emitted
after the next pipeline's phase matmuls; the next initial state prefetches
under the current phases), so a reps=R NEFF has the same instruction count
as reps=16 and the harness delta measures steady-state device time per
pipeline iteration.
"""

import sys

sys.path.insert(0, "/opt/trn_rl_repo")

import numpy as np

import concourse.bacc as bacc
import concourse.mybir as mybir
from concourse.ap import AP
from concourse.bass_utils import run_bass_kernel_spmd
from concourse.tile import TileContext

NQ = 12
NL = 2
B = 512
NCORES = 8
BC = B // NCORES  # 64

F32 = mybir.dt.float32
F32R = mybir.dt.float32r
C128 = np.complex128

# ----------------------------------------------------------------------------
# Host-side math: gate matrices -> phase operators -> packed device tables
# ----------------------------------------------------------------------------


def _rx(t):
    c, s = np.cos(t / 2), np.sin(t / 2)
    return np.array([[c, -1j * s], [-1j * s, c]], dtype=C128)


def _rz(t):
    e = np.exp(-0.5j * t)
    return np.array([[e, 0], [0, np.conj(e)]], dtype=C128)


def _crx(t):
    m = np.eye(4, dtype=C128)
    m[2:, 2:] = _rx(t)
    return m


def _op_2q(G, qa, qb, n):
    dim = 2**n
    M = np.zeros((dim, dim), dtype=C128)
    sa, sb = 1 << (n - 1 - qa), 1 << (n - 1 - qb)
    for z in range(dim):
        a, b = (z // sa) % 2, (z // sb) % 2
        base = z - a * sa - b * sb
        for a2 in range(2):
            for b2 in range(2):
                M[base + a2 * sa + b2 * sb, z] += G[a2 * 2 + b2, a * 2 + b]
    return M


def _kron_list(mats):
    M = np.array([[1.0 + 0j]])
    for m in mats:
        M = np.kron(M, m)
    return M


def _build_tables(params):
    params = np.asarray(params, dtype=np.float64)
    oneq = [
        [_rz(params[l, NQ + q]) @ _rx(params[l, q]) for q in range(NQ)]
        for l in range(NL)
    ]
    lidx = np.arange(32)
    P0 = np.diag((1 - (lidx & 1)).astype(C128))
    P1 = np.diag((lidx & 1).astype(C128))
    hidx = np.arange(128)
    P0h = np.diag((1 - (hidx & 1)).astype(C128))
    P1h = np.diag((hidx & 1).astype(C128))

    def chainB(p):
        M = np.eye(32, dtype=C128)
        for (c, t), th in [((3, 4), p[1]), ((2, 3), p[2]), ((1, 2), p[3]),
                           ((0, 1), p[4])]:
            M = _op_2q(_crx(th), c, t, 5) @ M
        return M

    def chainA(p):
        M = np.eye(128, dtype=C128)
        for (c, t), th in [((5, 6), p[6]), ((4, 5), p[7]), ((3, 4), p[8]),
                           ((2, 3), p[9]), ((1, 2), p[10]), ((0, 1), p[11])]:
            M = _op_2q(_crx(th), c, t, 7) @ M
        return M

    phases = []
    for l in range(NL):
        p = params[l]
        c0, s0 = np.cos(p[0] / 2), np.sin(p[0] / 2)
        c5, s5 = np.cos(p[5] / 2), np.sin(p[5] / 2)
        cB = chainB(p)
        F1 = cB @ (P0 + c0 * P1)
        F2 = -1j * s0 * (cB @ P1)
        if l > 0:
            ol = _kron_list(oneq[l][7:])
            F1, F2 = F1 @ ol, F2 @ ol
        cA = chainA(p)
        E1 = cA @ (P0h + c5 * P1h)
        E2 = -1j * s5 * (cA @ P1h)
        if l + 1 < NL:
            oh = _kron_list(oneq[l + 1][:7])
            E1, E2 = oh @ E1, oh @ E2
        phases.append(("B", F1, F2))
        phases.append(("A", E1, E2))
    return oneq, phases


def _embed_factors(x, oneq):
    x = np.asarray(x, dtype=np.float64)
    nb = x.shape[0]
    u = np.empty((nb, NQ, 2), dtype=C128)
    for q in range(NQ):
        v = np.stack([np.cos(x[:, q] / 2), -1j * np.sin(x[:, q] / 2)], axis=1)
        u[:, q] = v @ oneq[0][q].T
    a = u[:, 0]
    for q in range(1, 7):
        a = np.einsum("bi,bj->bij", a, u[:, q]).reshape(nb, -1)
    c = u[:, 7]
    for q in range(8, 12):
        c = np.einsum("bi,bj->bij", c, u[:, q]).reshape(nb, -1)
    return a, c


def _bf(A, axis):
    """Butterfly (unnormalized Hadamard pairing) on bit-64 of a 128 axis."""
    s2 = np.sqrt(0.5)
    A = np.moveaxis(A, axis, 0)
    out = np.concatenate([(A[0:64] + A[64:128]) * s2,
                          (A[0:64] - A[64:128]) * s2], axis=0)
    return np.moveaxis(out, 0, axis)


def _shared_tables(params):
    """Phase + measurement tables (identical on all cores).

    Phases 0-2 are stored diagonalized in the Hadamard basis of their
    Xflip bit:  M1 (x) I + M2 (x) X  =  H (M1+M2 (+) M1-M2) H,  with every
    H factor folded into an adjacent table (rows = post-H of the previous
    phase, cols = pre-H of the next) or into s0 / the classic ph3 rows.
    The device then needs no flipped state copies for ph0-ph2."""
    _, phases = _build_tables(params)
    out = {}

    def kron4(M):
        return np.kron(M.T, np.eye(4))

    G = []
    for i, (side, M1, M2) in enumerate(phases):
        if i < 3:
            GP = kron4(M1 + M2) if side == "B" else (M1 + M2).T
            GM = kron4(M1 - M2) if side == "B" else (M1 - M2).T
            G.append([GP, GM])
        else:
            G1 = kron4(M1) if side == "B" else M1.T
            G2 = kron4(M2) if side == "B" else M2.T
            G.append([G1, G2])
    G[0] = [_bf(g, 1) for g in G[0]]                 # pre-H1 on cols
    G[1] = [_bf(_bf(g, 0), 1) for g in G[1]]         # post-H0 rows, pre-H2 cols
    G[2] = [_bf(g, 0) for g in G[2]]                 # post-H1 rows
    G[3] = [_bf(g, 0) for g in G[3]]                 # post-H2 rows

    for i, (Ga, Gb) in enumerate(G):
        R0 = np.concatenate([Ga.real, Ga.imag], 1)
        R1 = np.concatenate([-Ga.imag, Ga.real], 1)
        R2 = np.concatenate([Gb.real, Gb.imag], 1)
        R3 = np.concatenate([-Gb.imag, Gb.real], 1)
        out[f"ph{i}"] = np.ascontiguousarray(
            np.concatenate([R0, R1, R2, R3], 1), dtype=np.float32)

    V = np.zeros((128, 24), np.float32)
    lv, b4v = np.arange(128) // 4, np.arange(128) % 4
    for b4 in range(4):
        V[b4v == b4, b4] = 1.0
    for jq in range(5):
        sig = 1 - 2 * ((lv >> (4 - jq)) & 1)
        for b4 in range(4):
            V[b4v == b4, 4 + jq * 4 + b4] = sig[b4v == b4]
    W = np.zeros((128, 10), np.float32)
    hv = np.arange(128)
    for q in range(7):
        W[:, q] = 1 - 2 * ((hv >> (6 - q)) & 1)
    W[:, 8] = 1.0
    W[:, 9] = 1.0
    out["mV"], out["mW"] = V, W
    return out


def _core_tables(x_shard, oneq):
    """Per-core initial product state (64 samples) in B' layout:
    s0[l*4+b4, b16*128+h] with the (r|i) planes side by side."""
    a, c = _embed_factors(x_shard, oneq)
    full = a[:, :, None] * c[:, None, :]          # [64, h=128, l=32]
    arr = full.reshape(16, 4, 128, 32)            # [b16, b4, h, l]
    perm = arr.transpose(3, 1, 0, 2).reshape(128, 2048)
    s0 = np.empty((128, 4096), np.float32)
    s0[:, 0:2048] = perm.real
    s0[:, 2048:4096] = perm.imag
    # pre-H0: butterfly the passive h-bit-64 within each 128-col chunk
    v = s0.reshape(128, 2, 16, 2, 64)
    s2 = np.sqrt(0.5, dtype=np.float32)
    w = np.empty_like(v)
    w[:, :, :, 0, :] = (v[:, :, :, 0, :] + v[:, :, :, 1, :]) * s2
    w[:, :, :, 1, :] = (v[:, :, :, 0, :] - v[:, :, :, 1, :]) * s2
    s0 = np.ascontiguousarray(w.reshape(128, 4096))
    return {"s0": s0}


# ----------------------------------------------------------------------------
# Device program
# ----------------------------------------------------------------------------


def _evac_dst(tile, g):
    """[128,256] PSUM -> plane-split chunk g of a [128,4096] state tile."""
    return AP(tensor=tile.tensor, offset=g * 128,
              ap=[[4096, 128], [2048, 2], [1, 128]])


def _chunk(tile, g, plane):
    """Stationary AP for chunk g, plane 0/1 of a state tile."""
    off = plane * 2048 + g * 128
    return AP(tensor=tile.tensor, offset=off,
              ap=[[4096, 128], [1, 128]])


def _chunk_h(tile, g, plane, half):
    """64-col half of chunk g (the +/- Hadamard subspace)."""
    off = plane * 2048 + g * 128 + half * 64
    return AP(tensor=tile.tensor, offset=off,
              ap=[[4096, 128], [1, 64]])


def _flip_dst(tile, g):
    """Chunk g of a state tile, written at f-bit-64-flipped positions."""
    return AP(tensor=tile.tensor, offset=g * 128 + 64,
              ap=[[4096, 128], [2048, 2], [-64, 2], [1, 64]])


def build_nc(reps=1):
    nc = bacc.Bacc("TRN2", target_bir_lowering=False)
    d = {}
    for name, shape in [("s0", [128, 4096]),
                        ("ph0", [128, 1024]), ("ph1", [128, 1024]),
                        ("ph2", [128, 1024]), ("ph3", [128, 1024]),
                        ("mV", [128, 24]), ("mW", [128, 10])]:
        d[name] = nc.dram_tensor(name, shape, F32R, kind="ExternalInput")
    y_d = nc.dram_tensor("y", [BC, NQ], F32, kind="ExternalOutput")

    with TileContext(nc) as tc:
        with (
            tc.tile_pool(name="tabs", bufs=1) as tabs,
            tc.tile_pool(name="st", bufs=3) as stp,
            tc.tile_pool(name="s0p", bufs=2) as s0p,
            tc.tile_pool(name="sf", bufs=3) as sfp,
            tc.tile_pool(name="ms", bufs=2) as msp,
            tc.tile_pool(name="pp", bufs=4, space="PSUM") as pp,
            tc.tile_pool(name="pm", bufs=2, space="PSUM") as pm,
        ):
            tt = {}
            for name in ("ph0", "ph1", "ph2", "ph3", "mV", "mW"):
                tile = tabs.tile(list(d[name].shape), d[name].dtype, tag=name)
                nc.sync.dma_start(out=tile, in_=d[name].ap())
                tt[name] = tile

            # PSUM evacuation alternates the two PSUM-capable copy engines
            # (GPSIMD cannot access PSUM); flip copies run SBUF->SBUF on it
            evac_engs = (nc.scalar.copy, nc.vector.tensor_copy)

            # Warm the ACT function tables (Square + Copy) before the loop so
            # the act-table-load pass sees them loaded on every path into the
            # body and does not reload (1.3us) each iteration.
            warm = msp.tile([128, 16], F32, tag="warm", bufs=1)
            nc.scalar.activation(
                out=warm[:, 0:8], in_=tt["mW"][:, 0:8].bitcast(F32),
                func=mybir.ActivationFunctionType.Square)
            nc.scalar.copy(out=warm[:, 8:16], in_=tt["mW"][:, 0:8])

            def phases_part(ecnt):
                # ---- initial state: uneven plane-paired DMA segments (the
                # first covers just ph0's first chunk group) so the first
                # matmul starts early; S sits in a dedicated 2-buffer pool
                # whose previous buffer was last read 3 pipelines ago, so
                # the next pipeline's load prefetches under this one's
                # phases. ph0-ph2 are diagonal in the folded Hadamard basis
                # and need no flipped state; only ph3's Xflip term does.
                S = s0p.tile([128, 4096], F32R, tag="s0")
                segs = [(0, 256), (256, 768), (1024, 512), (1536, 512)]
                for off, width in segs:
                    seg = AP(tensor=S.tensor, offset=off,
                             ap=[[4096, 128], [2048, 2], [1, width]])
                    src_seg = AP(tensor=d["s0"], offset=off,
                                 ap=[[4096, 128], [2048, 2], [1, width]])
                    nc.sync.dma_start(out=seg, in_=src_seg)

                # ---- 4 phases
                Sf = None
                for pi in range(4):
                    tab = tt[f"ph{pi}"]
                    S2 = stp.tile([128, 4096], F32R, tag="st")
                    S2f = (sfp.tile([128, 4096], F32R, tag="sf",
                                    name=f"S2f_{pi}")
                           if pi == 2 else None)
                    for gp in range(8):
                        if pi < 3:
                            # Hadamard halves of the passive bit: + uses
                            # blk0/1, - uses blk2/3. PSUM matmul outputs
                            # must start at partition 0, so each half gets
                            # its own [64,512] tile; the two evacuations
                            # write partition ranges 0:64 / 64:128 of S2.
                            psP = pp.tile([64, 512], F32, tag="ppP",
                                          bufs=2)
                            psM = pp.tile([64, 512], F32, tag="ppM",
                                          bufs=2)
                            for half in range(2):
                                g = gp * 2 + half
                                for hb, ph in ((0, psP), (1, psM)):
                                    poh = ph[:, half * 256:(half + 1) * 256]
                                    nc.tensor.matmul(
                                        poh, _chunk_h(S, g, 0, hb),
                                        tab[:, hb * 512:hb * 512 + 256],
                                        start=True, stop=False)
                                    nc.tensor.matmul(
                                        poh, _chunk_h(S, g, 1, hb),
                                        tab[:, hb * 512 + 256:hb * 512 + 512],
                                        start=False, stop=True)
                            for hb, ph in ((0, psP), (1, psM)):
                                esrc = ph.rearrange("p (c a j) -> p c a j",
                                                    c=2, a=2)
                                edst = AP(tensor=S2.tensor,
                                          offset=hb * 64 * 4096 + gp * 256,
                                          ap=[[4096, 64], [128, 2],
                                              [2048, 2], [1, 128]])
                                evac_engs[ecnt % 2](out=edst, in_=esrc)
                                ecnt += 1
                            if S2f is not None:
                                for half in range(2):
                                    g = gp * 2 + half
                                    nc.gpsimd.tensor_copy(
                                        out=_flip_dst(S2f, g),
                                        in_=_evac_dst(S2, g))
                        else:
                            ps = pp.tile([128, 512], F32, tag="pp",
                                         bufs=2)
                            for half in range(2):
                                g = gp * 2 + half
                                po = ps[:, half * 256:(half + 1) * 256]
                                mms = [(_chunk(S, g, 0), tab[:, 0:256]),
                                       (_chunk(S, g, 1), tab[:, 256:512]),
                                       (_chunk(Sf, g, 0), tab[:, 512:768]),
                                       (_chunk(Sf, g, 1), tab[:, 768:1024])]
                                for mi, (st_ap, rh_ap) in enumerate(mms):
                                    nc.tensor.matmul(po, st_ap, rh_ap,
                                                     start=(mi == 0),
                                                     stop=(mi == 3))
                            esrc = ps.rearrange("p (c a j) -> p c a j",
                                                c=2, a=2)
                            edst = AP(tensor=S2.tensor, offset=gp * 256,
                                      ap=[[4096, 128], [128, 2], [2048, 2],
                                          [1, 128]])
                            evac_engs[ecnt % 2](out=edst, in_=esrc)
                            ecnt += 1
                            if S2f is not None:
                                for half in range(2):
                                    g = gp * 2 + half
                                    nc.gpsimd.tensor_copy(
                                        out=_flip_dst(S2f, g),
                                        in_=_evac_dst(S2, g))
                    S, Sf = S2, S2f
                return S, ecnt

            def square_part(S):
                # probs = re^2 + im^2 per 512-col block on ACT/DVE/Pool;
                # runs under the next pipeline's phase matmuls
                sq = msp.tile([128, 2048], F32R, tag="sq", name="sq_r")
                probs = msp.tile([128, 2048], F32R, tag="probs")
                for blk in range(4):
                    c0, c1 = blk * 512, (blk + 1) * 512
                    nc.scalar.activation(
                        out=sq[:, c0:c1],
                        in_=S[:, c0:c1].bitcast(F32),
                        func=mybir.ActivationFunctionType.Square)
                    nc.vector.tensor_mul(out=probs[:, c0:c1],
                                         in0=S[:, 2048 + c0:2048 + c1],
                                         in1=S[:, 2048 + c0:2048 + c1])
                    nc.gpsimd.tensor_add(out=probs[:, c0:c1],
                                         in0=probs[:, c0:c1],
                                         in1=sq[:, c0:c1])
                return probs

            def measure_part(probs, ecnt):
                # S1 column layout: [0:64] = sel block (g*4+b4),
                # [64*(j+1) : 64*(j+2)] = low-qubit j block (g*4+b4), j=0..4
                S1 = msp.tile([128, 384], F32R, tag="S1")
                ps2 = pm.tile([64, 24], F32, tag="pm2", bufs=1)
                # one PSUM bank, 4 rotating 24-col regions (region deps are
                # tracked per AP range, so matmul g+4 waits on evac g)
                psA = pm.tile([128, 96], F32, tag="pm1", bufs=1)
                for g in range(16):
                    reg = psA[:, (g % 4) * 24:(g % 4) * 24 + 24]
                    nc.tensor.matmul(reg, probs[:, g * 128:(g + 1) * 128],
                                     tt["mV"], start=True, stop=True)
                    # cols (j=sel,q7..q11; b4) -> S1[:, 64*j + g*4 + b4]
                    dall = S1.rearrange("p (j c) -> p j c", c=64)[
                        :, 0:6, g * 4:g * 4 + 4]
                    evac_engs[ecnt % 2](out=dall, in_=reg)
                    ecnt += 1
                # stage 2: everything lands as [b-rows, cols] in one PSUM
                nc.tensor.matmul(ps2[:, 0:8], S1[:, 0:64], tt["mW"][:, 0:8],
                                 start=True, stop=True)
                for j in range(5):
                    nc.tensor.matmul(ps2[:, 8 + 2 * j:10 + 2 * j],
                                     S1[:, 64 * (j + 1):64 * (j + 2)],
                                     tt["mW"][:, 8:10],
                                     start=True, stop=True)
                yt = msp.tile([64, 12], F32, tag="yt")
                nc.scalar.copy(out=yt[:, 0:7], in_=ps2[:, 0:7])
                lowsrc = ps2[:, 8:18].rearrange(
                    "p (a b) -> p a b", b=2)[:, :, 0:1]
                nc.vector.tensor_copy(out=yt[:, 7:12].unsqueeze(-1),
                                      in_=lowsrc)
                nc.sync.dma_start(out=y_d.ap(), in_=yt)
                return ecnt

            if reps == 1:
                S, ecnt = phases_part(0)
                measure_part(square_part(S), ecnt)
            else:
                # software-pipelined 16x-unrolled body: each pipeline's
                # measure matmuls are emitted after the NEXT pipeline's phase
                # matmuls so the PE never waits on square/evac chains, and
                # the loop barrier + first-pipeline DMA exposure amortize
                # over 16 pipelines
                assert reps % 16 == 0, "looped NEFF runs 16 pipelines per trip"
                with tc.For_i(0, reps // 16):
                    probs_prev = None
                    ecnt = 0
                    for k in range(16):
                        S, ecnt = phases_part(ecnt)
                        if probs_prev is not None:
                            ecnt = measure_part(probs_prev, ecnt)
                        probs_prev = square_part(S)
                    measure_part(probs_prev, ecnt)

    nc.compile()
    return nc


_NC_CACHE = {}


def _get_nc(reps=1):
    if reps not in _NC_CACHE:
        _NC_CACHE[reps] = build_nc(reps)
    return _NC_CACHE[reps]


def make_in_maps(x, params):
    oneq, _ = _build_tables(params)
    shared = _shared_tables(params)
    in_maps = []
    for core in range(NCORES):
        m = dict(shared)
        m.update(_core_tables(x[core * BC:(core + 1) * BC], oneq))
        in_maps.append(m)
    return in_maps


def kernel(x, params, _reps=1, _nc=None):
    x = np.asarray(x)
    params = np.asarray(params)
    nc = _nc if _nc is not None else _get_nc(_reps)
    in_maps = make_in_maps(x, params)
    res = run_bass_kernel_spmd(nc, in_maps, list(range(NCORES)))
    return np.concatenate(
        [res.results[c]["y"] for c in range(NCORES)], axis=0
    ).astype(np.float32)
